# revision 27
# baseline (speedup 1.0000x reference)
"""Bass/Trainium2 kernel for nn_GAT_25082609009415.

GAT: g = x[46,131072] @ W1[131072,2048] -> 8-head masked attention ->
ELU -> h @ W2[2048,64] -> 1-head attention -> mean -> MLP(46->12->1) -> sigmoid.

Strategy (8 NeuronCores), memory-roofline driven:
- K-shard the dominant GEMM: core c owns W1[16384c:16384(c+1), :].
- Quantize x and W1 to fp8-e4m3 on host (power-of-2 scales) -> 4x less
  HBM traffic than fp32; matmul in DoubleRow perf mode (2 k-tiles/instr).
- x is pre-transposed on host into k-tile-major lhsT layout (inner dim
  padded 46->48 for the DoubleRow 16B-step ISA rule): no on-chip
  transposes for the GEMM.
- Attention logit weights are folded into the GEMM on host: Wsrc/Wdst =
  einsum('khf,hf->kh', W1.reshape(K,8,256), a1-halves) appended as 16
  extra rhs columns. AllReduce payload is [g | esrc | edst] bf16.
- Tail runs transposed (eT[j,(h,i)]) so softmax sums and both attention
  matmuls need only one tiny PE transpose; ELU output lands directly in
  the k-tile-major layout the layer-2 GEMM wants.
- Warmup collective + warmup Exp during the GEMM hide the cc-stack
  cold start and the Act table load.
"""
import numpy as np

import concourse.bass as bass
import concourse.bacc as bacc
import concourse.tile as tile
from concourse import mybir
from concourse.bass_utils import run_bass_kernel_spmd

N = 46
KTOT = 131072
HID = 2048
HEADS = 8
F1 = HID // HEADS          # 256 features / head
OUTF = 64
NCORES = 8
KC = KTOT // NCORES        # 16384 contraction elems per core
KT = KC // 128             # 128 k-tiles per core
PAIRS = KT // 2            # 64 DoubleRow pairs per core
CHUNKS = [1, 2, 4, 8, 16, 16, 16, 1]  # pairs per W DMA chunk: small first
                           # chunks start the PE early, big middle chunks
                           # amortize DGE gaps (fewer engine-idle windows)
CPMAX = max(CHUNKS)
WCOL = HID + 16            # 2048 gemm cols + 8 Wsrc + 8 Wdst
KT2 = HID // 16 // 8       # 16 k-tiles for layer-2 GEMM
GCOL = HID + 16            # allreduce payload cols: g | esrc | edst
XP = 48                    # xT inner dim padded 46->48 (16B-step rule)
MASK_NEG = -1.0e4          # exp(<= -9900) == 0.0f exactly

F32 = mybir.dt.float32
BF16 = mybir.dt.bfloat16
F8 = mybir.dt.float8e4
AX = mybir.AxisListType
OP = mybir.AluOpType
ACTF = mybir.ActivationFunctionType
DR = mybir.MatmulPerfMode.DoubleRow


def build():
    nc = bacc.Bacc(
        "TRN2",
        target_bir_lowering=False,
        debug=False,
        enable_asserts=False,
        num_devices=NCORES,
    )
    xs = nc.dram_tensor("xs", [128, KT * XP], F8, kind="ExternalInput")
    wf = nc.dram_tensor("wf", [128, KT * WCOL], F8, kind="ExternalInput")
    w2b = nc.dram_tensor("w2b", [128, KT2 * (OUTF + 2)], BF16, kind="ExternalInput")
    adjmT = nc.dram_tensor("adjmT", [N, HEADS * N], F32, kind="ExternalInput")
    scl = nc.dram_tensor("scl", [128, 2], F32, kind="ExternalInput")
    ident = nc.dram_tensor("ident", [128, 128], F32, kind="ExternalInput")
    identb = nc.dram_tensor("identb", [128, 128], BF16, kind="ExternalInput")
    onesc = nc.dram_tensor("onesc", [128, 1], F32, kind="ExternalInput")
    onescb = nc.dram_tensor("onescb", [128, 1], BF16, kind="ExternalInput")
    mw1 = nc.dram_tensor("mw1", [N, 12], F32, kind="ExternalInput")
    mb1 = nc.dram_tensor("mb1", [1, 12], F32, kind="ExternalInput")
    mw2t = nc.dram_tensor("mw2t", [1, 12], F32, kind="ExternalInput")
    mb2n = nc.dram_tensor("mb2n", [1, 1], F32, kind="ExternalInput")
    out = nc.dram_tensor("out", [1, 1], F32, kind="ExternalOutput")

    with tile.TileContext(nc) as tc:
        with (
            tc.tile_pool(name="const", bufs=1) as cst,
            tc.tile_pool(name="sbx", bufs=1) as sbx,
            tc.tile_pool(name="sbw", bufs=2) as sbw,
            tc.tile_pool(name="sbt", bufs=1) as sbt,
            tc.tile_pool(name="dram", bufs=1, space="DRAM") as dram,
        ):
            # x (lhsT, host-transposed) first on the sync queue, ahead of
            # the W chunks; consts go on the scalar queue in parallel.
            xT = sbx.tile([128, PAIRS, 2, XP], F8, tag="xT")
            nc.sync.dma_start(xT[:], xs.ap())
            ident_sb = cst.tile([128, 128], F32, tag="ident")
            nc.scalar.dma_start(ident_sb[:], ident.ap())
            identb_sb = cst.tile([128, 128], BF16, tag="identb")
            nc.scalar.dma_start(identb_sb[:], identb.ap())
            onesc_sb = cst.tile([128, 1], F32, tag="onesc")
            nc.scalar.dma_start(onesc_sb[:], onesc.ap())
            onescb_sb = cst.tile([128, 1], BF16, tag="onescb")
            nc.scalar.dma_start(onescb_sb[:], onescb.ap())
            adjmT_sb = cst.tile([N, HEADS * N], F32, tag="adjmT")
            nc.scalar.dma_start(adjmT_sb[:], adjmT.ap())
            scl_sb = cst.tile([128, 2], F32, tag="scl")
            nc.scalar.dma_start(scl_sb[:], scl.ap())
            w2b_sb = cst.tile([128, KT2, OUTF + 2], BF16, tag="w2b")
            nc.scalar.dma_start(w2b_sb[:], w2b.ap())
            mw1_sb = cst.tile([N, 12], F32, tag="mw1")
            nc.scalar.dma_start(mw1_sb[:], mw1.ap())
            mb1_sb = cst.tile([1, 12], F32, tag="mb1")
            nc.scalar.dma_start(mb1_sb[:], mb1.ap())
            mw2t_sb = cst.tile([1, 12], F32, tag="mw2t")
            nc.scalar.dma_start(mw2t_sb[:], mw2t.ap())
            mb2n_sb = cst.tile([1, 1], F32, tag="mb2n")
            nc.scalar.dma_start(mb2n_sb[:], mb2n.ap())

            # warm the Act exp table while the GEMM streams (hides the
            # ~1.3us ACT_TABLE_LOAD off the tail's critical path)
            warm_sb = sbt.tile([1, 1], F32, tag="warm")
            nc.scalar.activation(warm_sb[:], scl_sb[0:1, 0:1], ACTF.Exp)

            # warm the collective stack with a tiny AllReduce (absorbs the
            # cc-core cold start while the GEMM runs; payload value unused)
            ccw_in = dram.tile([1, 1], F32, tag="ccwin")
            ccw_out = dram.tile([1, 1], F32, tag="ccwout")
            nc.gpsimd.collective_compute(
                "AllReduce",
                OP.add,
                replica_groups=[list(range(NCORES))],
                ins=[ccw_in[:].opt()],
                outs=[ccw_out[:].opt()],
            )

            # ---- main GEMM: stream W chunks, fp8 DoubleRow matmuls ----
            gp_sb = sbt.tile([N, GCOL], BF16, tag="gp")
            with tc.tile_pool(name="psA", bufs=1, space="PSUM") as psA:
                g_ps = psA.tile([N, HID], F32, tag="g")
                e_ps = psA.tile([N, 16], F32, tag="e")
                p = 0
                for ci, cp in enumerate(CHUNKS):
                    w = sbw.tile([128, CPMAX, 2, WCOL], F8, tag="w")
                    nc.sync.dma_start(
                        w[:, 0:cp, :, :],
                        wf.ap()[:, 2 * WCOL * p:2 * WCOL * (p + cp)],
                    )
                    for pp in range(cp):
                        st, sp = (p == 0), (p == PAIRS - 1)
                        lhs = xT[:, p, :, 0:N]
                        for nn in range(HID // 512):
                            nc.tensor.matmul(
                                g_ps[:, 512 * nn:512 * (nn + 1)],
                                lhs,
                                w[:, pp, :, 512 * nn:512 * (nn + 1)],
                                start=st,
                                stop=sp,
                                perf_mode=DR,
                            )
                        # esrc[i,h] | edst[j,h] from the 16 folded columns
                        nc.tensor.matmul(
                            e_ps[:],
                            lhs,
                            w[:, pp, :, HID:HID + 16],
                            start=st,
                            stop=sp,
                            perf_mode=DR,
                        )
                        p += 1
                # descale + bf16 cast, split across vector & scalar engines
                nc.vector.tensor_scalar(
                    gp_sb[:, 0:1024], g_ps[:, 0:1024], scl_sb[:N, 0:1], None,
                    OP.mult,
                )
                nc.scalar.activation(
                    gp_sb[:, 1024:HID], g_ps[:, 1024:HID], ACTF.Identity,
                    scale=scl_sb[:N, 0:1],
                )
                nc.vector.tensor_scalar(
                    gp_sb[:, HID:GCOL], e_ps[:], scl_sb[:N, 1:2], None, OP.mult
                )

            # ---- AllReduce partial [g | esrc | edst] over the 8 cores ----
            # in/out DMAs issued from the gpsimd queue so the collective
            # trigger follows its input DMA with no cross-engine handoff
            # split the small cc transfers across two HWDGE queues — a
            # single 46-descriptor DMA only engages 2 of 16 DMA engines
            cc_in = dram.tile([N, GCOL], BF16, tag="ccin")
            cc_out = dram.tile([N, GCOL], BF16, tag="ccout")
            nc.sync.dma_start(cc_in[0:23, :], gp_sb[0:23, :])
            nc.scalar.dma_start(cc_in[23:N, :], gp_sb[23:N, :])
            nc.gpsimd.collective_compute(
                "AllReduce",
                OP.add,
                replica_groups=[list(range(NCORES))],
                ins=[cc_in[:].opt()],
                outs=[cc_out[:].opt()],
            )
            g_sb = sbt.tile([N, GCOL], BF16, tag="gsb")
            nc.sync.dma_start(g_sb[0:23, :], cc_out[0:23, :])
            nc.scalar.dma_start(g_sb[23:N, :], cc_out[23:N, :])

            with (
                tc.tile_pool(name="psR", bufs=2, space="PSUM") as psR,
                tc.tile_pool(name="psH", bufs=1, space="PSUM") as psH,
                tc.tile_pool(name="psO", bufs=2, space="PSUM") as psO,
            ):
                # ---- layer-1 attention, transposed: eT[j, (h,i)] ----
                # esrcT[h, i] <- PE-transpose of the esrc cols, then an
                # esrc row [1,(h,i)] via 8 selector matmuls, broadcast down
                esT_ps = psR.tile([8, N], BF16, tag="row")
                nc.tensor.transpose(
                    esT_ps[:], g_sb[:, HID:HID + 8], identb_sb[:N, :N]
                )
                esT_sb = sbt.tile([8, N], F32, tag="esT")
                nc.vector.tensor_copy(esT_sb[:], esT_ps[:])
                esr_ps = psR.tile([1, HEADS * N], F32, tag="row")
                for h in range(HEADS):
                    nc.tensor.matmul(
                        esr_ps[0:1, N * h:N * (h + 1)],
                        ident_sb[0:8, h:h + 1],
                        esT_sb[:],
                        start=True,
                        stop=True,
                    )
                esr_sb = sbt.tile([1, HEADS * N], F32, tag="esr")
                nc.vector.tensor_copy(esr_sb[:], esr_ps[:])
                ebase_sb = sbt.tile([N, HEADS * N], F32, tag="ebase")
                nc.gpsimd.partition_broadcast(ebase_sb[:], esr_sb[:])
                ed_sb = sbt.tile([N, 8], F32, tag="ed")
                nc.scalar.activation(
                    ed_sb[:], g_sb[:, HID + 8:GCOL], ACTF.Identity
                )
                # mask + edst folded off the critical chain (runs while the
                # PE builds the esrc row); leaky(e)+mask == leaky(e+mask)
                # here since -1e4 and -2e3 both exp() to exactly 0
                med_sb = sbt.tile([N, HEADS, N], F32, tag="med")
                nc.vector.tensor_add(
                    med_sb[:],
                    adjmT_sb[:].rearrange("p (h i) -> p h i", h=HEADS),
                    ed_sb[:].unsqueeze(2).broadcast_to([N, HEADS, N]),
                )

                eT_sb = sbt.tile([N, HEADS, N], F32, tag="eT")
                nc.vector.tensor_add(
                    eT_sb[:],
                    ebase_sb[:].rearrange("p (h i) -> p h i", h=HEADS),
                    med_sb[:],
                )
                nc.vector.scalar_tensor_tensor(
                    eT_sb[:], eT_sb[:], 0.2, eT_sb[:], op0=OP.mult, op1=OP.max
                )
                uT_sb = sbt.tile([N, HEADS * N], F32, tag="uT")
                nc.scalar.activation(uT_sb[:], eT_sb[:], ACTF.Exp)

                s_ps = psR.tile([1, HEADS * N], F32, tag="row")
                nc.tensor.matmul(
                    s_ps[:], onesc_sb[:N, 0:1], uT_sb[:], start=True, stop=True
                )
                r_sb = sbt.tile([1, HEADS * N], F32, tag="r")
                nc.vector.reciprocal_approx_fast(r_sb[:], s_ps[:])
                rbc_sb = sbt.tile([N, HEADS * N], F32, tag="rbc")
                nc.gpsimd.partition_broadcast(rbc_sb[:], r_sb[:])
                attT_sb = sbt.tile([N, HEADS * N], BF16, tag="attT")
                nc.vector.tensor_mul(attT_sb[:], uT_sb[:], rbc_sb[:])

                # h1T[f, i] per k-tile: lands k-tile-major for layer 2
                hta = psH.tile([128, 8, N], F32, tag="hta")
                htb = psH.tile([128, 8, N], F32, tag="htb")
                hts = [hta, htb]
                for k in range(KT2):
                    h = k // 2
                    nc.tensor.matmul(
                        hts[k // 8][:, k % 8, :],
                        g_sb[:, 128 * k:128 * (k + 1)],
                        attT_sb[:, N * h:N * (h + 1)],
                        start=True,
                        stop=True,
                    )
                # ELU -> bf16, still k-tile-major
                hT_sb = sbt.tile([128, KT2, N], BF16, tag="hT")
                for half in range(2):
                    src = hts[half][:]
                    tneg = sbt.tile([128, 8, N], F32, tag="tneg")
                    nc.vector.tensor_scalar_min(tneg[:], src, 0.0)
                    texp = sbt.tile([128, 8, N], F32, tag="texp")
                    nc.scalar.activation(texp[:], tneg[:], ACTF.Exp)
                    tpos = sbt.tile([128, 8, N], F32, tag="tpos")
                    nc.vector.tensor_scalar_max(tpos[:], src, 0.0)
                    nc.vector.scalar_tensor_tensor(
                        hT_sb[:, 8 * half:8 * (half + 1), :],
                        texp[:], -1.0, tpos[:], op0=OP.add, op1=OP.add,
                    )

                # ---- layer-2 GEMM (+ folded e2 cols) ----
                g2_ps = psO.tile([N, OUTF + 2], F32, tag="o")
                for k in range(KT2):
                    nc.tensor.matmul(
                        g2_ps[:],
                        hT_sb[:, k, :],
                        w2b_sb[:, k, :],
                        start=(k == 0),
                        stop=(k == KT2 - 1),
                    )
                g2_sb = sbt.tile([N, OUTF], BF16, tag="g2")
                nc.vector.tensor_copy(g2_sb[:], g2_ps[:, 0:OUTF])
                e2c_sb = sbt.tile([N, 2], F32, tag="e2c")
                nc.vector.tensor_copy(e2c_sb[:], g2_ps[:, OUTF:OUTF + 2])

                # ---- layer-2 attention, transposed: e2T[j, i] ----
                e2sT_ps = psR.tile([1, N], F32, tag="row")
                nc.tensor.matmul(
                    e2sT_ps[:], e2c_sb[:, 0:1], ident_sb[:N, :N],
                    start=True, stop=True,
                )
                e2sT_sb = sbt.tile([1, N], F32, tag="e2sT")
                nc.vector.tensor_copy(e2sT_sb[:], e2sT_ps[:])
                e2b_sb = sbt.tile([N, N], F32, tag="e2b")
                nc.gpsimd.partition_broadcast(e2b_sb[:], e2sT_sb[:])
                # mask + e2d folded off the critical chain (same leaky
                # commute as layer 1)
                med2_sb = sbt.tile([N, N], F32, tag="med2")
                nc.vector.tensor_scalar(
                    med2_sb[:], adjmT_sb[:, 0:N], e2c_sb[:, 1:2], None, OP.add
                )
                e2_sb = sbt.tile([N, N], F32, tag="e2")
                nc.vector.tensor_add(e2_sb[:], e2b_sb[:], med2_sb[:])
                nc.vector.scalar_tensor_tensor(
                    e2_sb[:], e2_sb[:], 0.2, e2_sb[:], op0=OP.mult, op1=OP.max
                )
                u2T_sb = sbt.tile([N, N], BF16, tag="u2T")
                nc.scalar.activation(u2T_sb[:], e2_sb[:], ACTF.Exp)

                s2_ps = psR.tile([N, 1], F32, tag="row")
                nc.tensor.matmul(
                    s2_ps[:], u2T_sb[:], onescb_sb[:N, 0:1], start=True, stop=True
                )
                r2_sb = sbt.tile([N, 1], F32, tag="r2")
                nc.vector.reciprocal_approx_fast(r2_sb[:], s2_ps[:])

                # o2T = g2^T @ u2T (unnormalized); r2 folded into m below
                o2T_ps = psO.tile([OUTF, N], F32, tag="o")
                nc.tensor.matmul(
                    o2T_ps[:], g2_sb[:], u2T_sb[:], start=True, stop=True
                )
                o2T_sb = sbt.tile([OUTF, N], F32, tag="o2T")
                nc.vector.tensor_copy(o2T_sb[:], o2T_ps[:])
                m_ps = psO.tile([N, 1], F32, tag="o")
                nc.tensor.matmul(
                    m_ps[:], o2T_sb[:], onesc_sb[:OUTF, 0:1], start=True, stop=True
                )
                m_sb = sbt.tile([N, 1], F32, tag="m")
                nc.vector.tensor_scalar(
                    m_sb[:], m_ps[:], r2_sb[:, 0:1], None, OP.mult
                )

                # ---- MLP head (mw1 pre-divided by 64 on host) ----
                z1_ps = psO.tile([1, 12], F32, tag="o")
                nc.tensor.matmul(z1_ps[:], m_sb[:], mw1_sb[:], start=True, stop=True)
                z1_sb = sbt.tile([1, 12], F32, tag="z1")
                nc.vector.tensor_add(z1_sb[:], z1_ps[:], mb1_sb[:])
                zt_sb = sbt.tile([1, 12], F32, tag="zt")
                nc.vector.tensor_mul(zt_sb[:], z1_sb[:], mw2t_sb[:])
                z2_sb = sbt.tile([1, 1], F32, tag="z2")
                nc.vector.tensor_reduce(z2_sb[:], zt_sb[:], axis=AX.X, op=OP.add)
                # sigmoid(z + b) = 1/(1 + exp(-z - b)): keeps the Exp act
                # table loaded (no Sigmoid table switch)
                ez_sb = sbt.tile([1, 1], F32, tag="ez")
                nc.scalar.activation(
                    ez_sb[:], z2_sb[:], ACTF.Exp, bias=mb2n_sb[:, 0:1],
                    scale=-1.0,
                )
                ez1_sb = sbt.tile([1, 1], F32, tag="ez1")
                nc.vector.tensor_scalar(ez1_sb[:], ez_sb[:], 1.0, None, OP.add)
                res_sb = sbt.tile([1, 1], F32, tag="res")
                nc.vector.reciprocal(res_sb[:], ez1_sb[:])
                nc.sync.dma_start(out.ap(), res_sb[:])

    nc.compile()
    return nc


_NC_CACHE = []


def _get_nc():
    if not _NC_CACHE:
        _NC_CACHE.append(build())
    return _NC_CACHE[0]


def _q8(a):
    import ml_dtypes

    return np.clip(a, -240.0, 240.0).astype(ml_dtypes.float8_e4m3)


def _prep_in_maps(x, adj, W1, a1, W2, a2, mw1, mb1, mw2, mb2):
    import ml_dtypes

    # host folds: attention logit weights as extra GEMM columns
    W1r = W1.reshape(KTOT, HEADS, F1)
    Wsrc = np.einsum("khf,hf->kh", W1r, a1[:, :F1]).astype(np.float32)
    Wdst = np.einsum("khf,hf->kh", W1r, a1[:, F1:]).astype(np.float32)
    sx = 2.0 ** np.floor(np.log2(224.0 / np.abs(x).max()))
    sW = 2.0 ** np.floor(np.log2(224.0 / np.abs(W1).max()))
    sA = 2.0 ** np.floor(
        np.log2(224.0 / max(np.abs(Wsrc).max(), np.abs(Wdst).max()))
    )
    scl = np.zeros((128, 2), np.float32)
    scl[:, 0] = 1.0 / (sx * sW)
    scl[:, 1] = 1.0 / (sx * sA)

    wq = _q8(
        np.concatenate([W1 * sW, Wsrc * sA, Wdst * sA], axis=1)
    )  # [131072, 2064] fp8
    xq = _q8(x * sx)  # [46, 131072] fp8

    mask = np.where(adj[:, :, 0], np.float32(0.0), np.float32(MASK_NEG))
    adjmT = np.ascontiguousarray(np.tile(mask.T, (1, HEADS)))  # [46, 368]

    a2s, a2d = a2[0, :OUTF], a2[0, OUTF:]
    w2full = np.concatenate(
        [W2, (W2 @ a2s)[:, None], (W2 @ a2d)[:, None]], axis=1
    ).astype(ml_dtypes.bfloat16)  # [2048, 66]
    w2r = np.ascontiguousarray(
        w2full.reshape(KT2, 128, OUTF + 2).transpose(1, 0, 2).reshape(128, -1)
    )

    shared = {
        "w2b": w2r,
        "adjmT": adjmT,
        "scl": scl,
        "ident": np.eye(128, dtype=np.float32),
        "identb": np.eye(128, dtype=ml_dtypes.bfloat16),
        "onesc": np.ones((128, 1), np.float32),
        "onescb": np.ones((128, 1), ml_dtypes.bfloat16),
        "mw1": np.ascontiguousarray(mw1 / np.float32(OUTF)),
        "mb1": mb1.reshape(1, 12).astype(np.float32),
        "mw2t": np.ascontiguousarray(mw2.reshape(1, 12)),
        "mb2n": (-mb2).reshape(1, 1).astype(np.float32),
    }
    in_maps = []
    for c in range(NCORES):
        m = dict(shared)
        xc = xq[:, KC * c:KC * (c + 1)]  # [46, 16384]
        xcT = np.zeros((KT, 128, XP), xq.dtype)
        xcT[:, :, :N] = xc.T.reshape(KT, 128, N)
        m["xs"] = np.ascontiguousarray(
            xcT.transpose(1, 0, 2).reshape(128, KT * XP)
        )
        wc = wq[KC * c:KC * (c + 1), :]  # [16384, 2064]
        m["wf"] = np.ascontiguousarray(
            wc.reshape(KT, 128, WCOL).transpose(1, 0, 2).reshape(128, KT * WCOL)
        )
        in_maps.append(m)
    return in_maps


def kernel(**inputs):
    x = np.asarray(inputs["x"], dtype=np.float32)
    adj = np.asarray(inputs["adj_mat"]).astype(bool).reshape(N, N, 1)
    W1 = np.asarray(inputs["W1"], dtype=np.float32)
    a1 = np.asarray(inputs["a1"], dtype=np.float32)
    W2 = np.asarray(inputs["W2"], dtype=np.float32)
    a2 = np.asarray(inputs["a2"], dtype=np.float32)
    mw1 = np.asarray(inputs["mlp_w1"], dtype=np.float32)
    mb1 = np.asarray(inputs["mlp_b1"], dtype=np.float32)
    mw2 = np.asarray(inputs["mlp_w2"], dtype=np.float32)
    mb2 = np.asarray(inputs["mlp_b2"], dtype=np.float32)

    nc = _get_nc()
    in_maps = _prep_in_maps(x, adj, W1, a1, W2, a2, mw1, mb1, mw2, mb2)
    res = run_bass_kernel_spmd(nc, in_maps, core_ids=list(range(NCORES)))
    return res.results[0]["out"].reshape(1).astype(np.float32)


# revision 28
# speedup vs baseline: 1.0534x; 1.0534x over previous
"""Bass/Trainium2 kernel for nn_GAT_25082609009415.

GAT: g = x[46,131072] @ W1[131072,2048] -> 8-head masked attention ->
ELU -> h @ W2[2048,64] -> 1-head attention -> mean -> MLP(46->12->1) -> sigmoid.

Strategy (8 NeuronCores), memory-roofline driven:
- K-shard the dominant GEMM: core c owns W1[16384c:16384(c+1), :].
- Quantize x and W1 to fp8-e4m3 on host (power-of-2 scales) -> 4x less
  HBM traffic than fp32; matmul in DoubleRow perf mode (2 k-tiles/instr).
- x is pre-transposed on host into k-tile-major lhsT layout (inner dim
  padded 46->48 for the DoubleRow 16B-step ISA rule): no on-chip
  transposes for the GEMM.
- Attention logit weights are folded into the GEMM on host: Wsrc/Wdst =
  einsum('khf,hf->kh', W1.reshape(K,8,256), a1-halves) appended as 16
  extra rhs columns. AllReduce payload is [g | esrc | edst] bf16.
- Tail runs transposed (eT[j,(h,i)]) so softmax sums and both attention
  matmuls need only one tiny PE transpose; ELU output lands directly in
  the k-tile-major layout the layer-2 GEMM wants.
- Warmup collective + warmup Exp during the GEMM hide the cc-stack
  cold start and the Act table load.
"""
import numpy as np

import concourse.bass as bass
import concourse.bacc as bacc
import concourse.tile as tile
from concourse import mybir
from concourse.bass_utils import run_bass_kernel_spmd

N = 46
KTOT = 131072
HID = 2048
HEADS = 8
F1 = HID // HEADS          # 256 features / head
OUTF = 64
NCORES = 8
KC = KTOT // NCORES        # 16384 contraction elems per core
KT = KC // 128             # 128 k-tiles per core
PAIRS = KT // 2            # 64 DoubleRow pairs per core
CHUNKS = [2, 6] + [8] * 7  # pairs per W DMA chunk (small first chunk
                           # starts the PE early; uniform stream after)
CPMAX = max(CHUNKS)
WCOL = HID + 16            # 2048 gemm cols + 8 Wsrc + 8 Wdst
KT2 = HID // 16 // 8       # 16 k-tiles for layer-2 GEMM
GCOL = HID + 16            # allreduce payload cols: g | esrc | edst
XP = 48                    # xT inner dim padded 46->48 (16B-step rule)
MASK_NEG = -1.0e4          # exp(<= -9900) == 0.0f exactly

F32 = mybir.dt.float32
BF16 = mybir.dt.bfloat16
F8 = mybir.dt.float8e4
AX = mybir.AxisListType
OP = mybir.AluOpType
ACTF = mybir.ActivationFunctionType
DR = mybir.MatmulPerfMode.DoubleRow


def build():
    nc = bacc.Bacc(
        "TRN2",
        target_bir_lowering=False,
        debug=False,
        enable_asserts=False,
        num_devices=NCORES,
    )
    xs = nc.dram_tensor("xs", [128, KT * XP], F8, kind="ExternalInput")
    wf = nc.dram_tensor("wf", [128, KT * WCOL], F8, kind="ExternalInput")
    w2b = nc.dram_tensor("w2b", [128, KT2 * (OUTF + 2)], BF16, kind="ExternalInput")
    adjmT = nc.dram_tensor("adjmT", [N, HEADS * N], F32, kind="ExternalInput")
    scl = nc.dram_tensor("scl", [128, 2], F32, kind="ExternalInput")
    ident = nc.dram_tensor("ident", [128, 128], F32, kind="ExternalInput")
    identb = nc.dram_tensor("identb", [128, 128], BF16, kind="ExternalInput")
    onesc = nc.dram_tensor("onesc", [128, 1], F32, kind="ExternalInput")
    onescb = nc.dram_tensor("onescb", [128, 1], BF16, kind="ExternalInput")
    mw1 = nc.dram_tensor("mw1", [N, 12], F32, kind="ExternalInput")
    mb1 = nc.dram_tensor("mb1", [1, 12], F32, kind="ExternalInput")
    mw2t = nc.dram_tensor("mw2t", [1, 12], F32, kind="ExternalInput")
    mb2n = nc.dram_tensor("mb2n", [1, 1], F32, kind="ExternalInput")
    out = nc.dram_tensor("out", [1, 1], F32, kind="ExternalOutput")

    with tile.TileContext(nc) as tc:
        with (
            tc.tile_pool(name="const", bufs=1) as cst,
            tc.tile_pool(name="sbx", bufs=1) as sbx,
            tc.tile_pool(name="sbw", bufs=2) as sbw,
            tc.tile_pool(name="sbt", bufs=1) as sbt,
            tc.tile_pool(name="dram", bufs=1, space="DRAM") as dram,
        ):
            # x (lhsT, host-transposed) first on the sync queue, ahead of
            # the W chunks; consts go on the scalar queue in parallel.
            xT = sbx.tile([128, PAIRS, 2, XP], F8, tag="xT")
            nc.sync.dma_start(xT[:], xs.ap())
            ident_sb = cst.tile([128, 128], F32, tag="ident")
            nc.scalar.dma_start(ident_sb[:], ident.ap())
            identb_sb = cst.tile([128, 128], BF16, tag="identb")
            nc.scalar.dma_start(identb_sb[:], identb.ap())
            onesc_sb = cst.tile([128, 1], F32, tag="onesc")
            nc.scalar.dma_start(onesc_sb[:], onesc.ap())
            onescb_sb = cst.tile([128, 1], BF16, tag="onescb")
            nc.scalar.dma_start(onescb_sb[:], onescb.ap())
            adjmT_sb = cst.tile([N, HEADS * N], F32, tag="adjmT")
            nc.scalar.dma_start(adjmT_sb[:], adjmT.ap())
            scl_sb = cst.tile([128, 2], F32, tag="scl")
            nc.scalar.dma_start(scl_sb[:], scl.ap())
            w2b_sb = cst.tile([128, KT2, OUTF + 2], BF16, tag="w2b")
            nc.scalar.dma_start(w2b_sb[:], w2b.ap())
            mw1_sb = cst.tile([N, 12], F32, tag="mw1")
            nc.scalar.dma_start(mw1_sb[:], mw1.ap())
            mb1_sb = cst.tile([1, 12], F32, tag="mb1")
            nc.scalar.dma_start(mb1_sb[:], mb1.ap())
            mw2t_sb = cst.tile([1, 12], F32, tag="mw2t")
            nc.scalar.dma_start(mw2t_sb[:], mw2t.ap())
            mb2n_sb = cst.tile([1, 1], F32, tag="mb2n")
            nc.scalar.dma_start(mb2n_sb[:], mb2n.ap())

            # warm the Act exp table while the GEMM streams (hides the
            # ~1.3us ACT_TABLE_LOAD off the tail's critical path)
            warm_sb = sbt.tile([1, 1], F32, tag="warm")
            nc.scalar.activation(warm_sb[:], scl_sb[0:1, 0:1], ACTF.Exp)

            # warm the collective stack with a tiny AllReduce (absorbs the
            # cc-core cold start while the GEMM runs; payload value unused)
            ccw_in = dram.tile([1, 1], F32, tag="ccwin")
            ccw_out = dram.tile([1, 1], F32, tag="ccwout")
            nc.gpsimd.collective_compute(
                "AllReduce",
                OP.add,
                replica_groups=[list(range(NCORES))],
                ins=[ccw_in[:].opt()],
                outs=[ccw_out[:].opt()],
            )

            # ---- main GEMM: stream W chunks, fp8 DoubleRow matmuls ----
            gp_sb = sbt.tile([N, GCOL], BF16, tag="gp")
            with tc.tile_pool(name="psA", bufs=1, space="PSUM") as psA:
                g_ps = psA.tile([N, HID], F32, tag="g")
                e_ps = psA.tile([N, 16], F32, tag="e")
                p = 0
                for ci, cp in enumerate(CHUNKS):
                    w = sbw.tile([128, CPMAX, 2, WCOL], F8, tag="w")
                    nc.sync.dma_start(
                        w[:, 0:cp, :, :],
                        wf.ap()[:, 2 * WCOL * p:2 * WCOL * (p + cp)],
                    )
                    for pp in range(cp):
                        st, sp = (p == 0), (p == PAIRS - 1)
                        lhs = xT[:, p, :, 0:N]
                        for nn in range(HID // 512):
                            nc.tensor.matmul(
                                g_ps[:, 512 * nn:512 * (nn + 1)],
                                lhs,
                                w[:, pp, :, 512 * nn:512 * (nn + 1)],
                                start=st,
                                stop=sp,
                                perf_mode=DR,
                            )
                        # esrc[i,h] | edst[j,h] from the 16 folded columns
                        nc.tensor.matmul(
                            e_ps[:],
                            lhs,
                            w[:, pp, :, HID:HID + 16],
                            start=st,
                            stop=sp,
                            perf_mode=DR,
                        )
                        p += 1
                # descale + bf16 cast, split across vector & scalar engines
                nc.vector.tensor_scalar(
                    gp_sb[:, 0:1024], g_ps[:, 0:1024], scl_sb[:N, 0:1], None,
                    OP.mult,
                )
                nc.scalar.activation(
                    gp_sb[:, 1024:HID], g_ps[:, 1024:HID], ACTF.Identity,
                    scale=scl_sb[:N, 0:1],
                )
                nc.vector.tensor_scalar(
                    gp_sb[:, HID:GCOL], e_ps[:], scl_sb[:N, 1:2], None, OP.mult
                )

            # ---- AllReduce partial [g | esrc | edst] over the 8 cores ----
            # in/out DMAs issued from the gpsimd queue so the collective
            # trigger follows its input DMA with no cross-engine handoff
            # split the small cc transfers across two HWDGE queues — a
            # single 46-descriptor DMA only engages 2 of 16 DMA engines
            cc_in = dram.tile([N, GCOL], BF16, tag="ccin")
            cc_out = dram.tile([N, GCOL], BF16, tag="ccout")
            nc.sync.dma_start(cc_in[0:23, :], gp_sb[0:23, :])
            nc.scalar.dma_start(cc_in[23:N, :], gp_sb[23:N, :])
            nc.gpsimd.collective_compute(
                "AllReduce",
                OP.add,
                replica_groups=[list(range(NCORES))],
                ins=[cc_in[:].opt()],
                outs=[cc_out[:].opt()],
            )
            g_sb = sbt.tile([N, GCOL], BF16, tag="gsb")
            nc.sync.dma_start(g_sb[0:23, :], cc_out[0:23, :])
            nc.scalar.dma_start(g_sb[23:N, :], cc_out[23:N, :])

            with (
                tc.tile_pool(name="psR", bufs=2, space="PSUM") as psR,
                tc.tile_pool(name="psH", bufs=1, space="PSUM") as psH,
                tc.tile_pool(name="psO", bufs=2, space="PSUM") as psO,
            ):
                # ---- layer-1 attention, transposed: eT[j, (h,i)] ----
                # esrcT[h, i] <- PE-transpose of the esrc cols, then an
                # esrc row [1,(h,i)] via 8 selector matmuls, broadcast down
                esT_ps = psR.tile([8, N], BF16, tag="row")
                nc.tensor.transpose(
                    esT_ps[:], g_sb[:, HID:HID + 8], identb_sb[:N, :N]
                )
                esT_sb = sbt.tile([8, N], F32, tag="esT")
                nc.vector.tensor_copy(esT_sb[:], esT_ps[:])
                esr_ps = psR.tile([1, HEADS * N], F32, tag="row")
                for h in range(HEADS):
                    nc.tensor.matmul(
                        esr_ps[0:1, N * h:N * (h + 1)],
                        ident_sb[0:8, h:h + 1],
                        esT_sb[:],
                        start=True,
                        stop=True,
                    )
                esr_sb = sbt.tile([1, HEADS * N], F32, tag="esr")
                nc.vector.tensor_copy(esr_sb[:], esr_ps[:])
                ebase_sb = sbt.tile([N, HEADS * N], F32, tag="ebase")
                nc.gpsimd.partition_broadcast(ebase_sb[:], esr_sb[:])
                ed_sb = sbt.tile([N, 8], F32, tag="ed")
                nc.scalar.activation(
                    ed_sb[:], g_sb[:, HID + 8:GCOL], ACTF.Identity
                )
                # mask + edst folded off the critical chain (runs while the
                # PE builds the esrc row); leaky(e)+mask == leaky(e+mask)
                # here since -1e4 and -2e3 both exp() to exactly 0
                med_sb = sbt.tile([N, HEADS, N], F32, tag="med")
                nc.vector.tensor_add(
                    med_sb[:],
                    adjmT_sb[:].rearrange("p (h i) -> p h i", h=HEADS),
                    ed_sb[:].unsqueeze(2).broadcast_to([N, HEADS, N]),
                )

                eT_sb = sbt.tile([N, HEADS, N], F32, tag="eT")
                nc.vector.tensor_add(
                    eT_sb[:],
                    ebase_sb[:].rearrange("p (h i) -> p h i", h=HEADS),
                    med_sb[:],
                )
                nc.vector.scalar_tensor_tensor(
                    eT_sb[:], eT_sb[:], 0.2, eT_sb[:], op0=OP.mult, op1=OP.max
                )
                uT_sb = sbt.tile([N, HEADS * N], F32, tag="uT")
                nc.scalar.activation(uT_sb[:], eT_sb[:], ACTF.Exp)

                s_ps = psR.tile([1, HEADS * N], F32, tag="row")
                nc.tensor.matmul(
                    s_ps[:], onesc_sb[:N, 0:1], uT_sb[:], start=True, stop=True
                )
                r_sb = sbt.tile([1, HEADS * N], F32, tag="r")
                nc.vector.reciprocal_approx_fast(r_sb[:], s_ps[:])
                rbc_sb = sbt.tile([N, HEADS * N], F32, tag="rbc")
                nc.gpsimd.partition_broadcast(rbc_sb[:], r_sb[:])
                attT_sb = sbt.tile([N, HEADS * N], BF16, tag="attT")
                nc.vector.tensor_mul(attT_sb[:], uT_sb[:], rbc_sb[:])

                # h1T[f, i] per k-tile: lands k-tile-major for layer 2
                hta = psH.tile([128, 8, N], F32, tag="hta")
                htb = psH.tile([128, 8, N], F32, tag="htb")
                hts = [hta, htb]
                for k in range(KT2):
                    h = k // 2
                    nc.tensor.matmul(
                        hts[k // 8][:, k % 8, :],
                        g_sb[:, 128 * k:128 * (k + 1)],
                        attT_sb[:, N * h:N * (h + 1)],
                        start=True,
                        stop=True,
                    )
                # ELU -> bf16, still k-tile-major
                hT_sb = sbt.tile([128, KT2, N], BF16, tag="hT")
                for half in range(2):
                    src = hts[half][:]
                    tneg = sbt.tile([128, 8, N], F32, tag="tneg")
                    nc.vector.tensor_scalar_min(tneg[:], src, 0.0)
                    texp = sbt.tile([128, 8, N], F32, tag="texp")
                    nc.scalar.activation(texp[:], tneg[:], ACTF.Exp)
                    tpos = sbt.tile([128, 8, N], F32, tag="tpos")
                    nc.vector.tensor_scalar_max(tpos[:], src, 0.0)
                    nc.vector.scalar_tensor_tensor(
                        hT_sb[:, 8 * half:8 * (half + 1), :],
                        texp[:], -1.0, tpos[:], op0=OP.add, op1=OP.add,
                    )

                # ---- layer-2 GEMM (+ folded e2 cols) ----
                g2_ps = psO.tile([N, OUTF + 2], F32, tag="o")
                for k in range(KT2):
                    nc.tensor.matmul(
                        g2_ps[:],
                        hT_sb[:, k, :],
                        w2b_sb[:, k, :],
                        start=(k == 0),
                        stop=(k == KT2 - 1),
                    )
                g2_sb = sbt.tile([N, OUTF], BF16, tag="g2")
                nc.vector.tensor_copy(g2_sb[:], g2_ps[:, 0:OUTF])
                e2c_sb = sbt.tile([N, 2], F32, tag="e2c")
                nc.vector.tensor_copy(e2c_sb[:], g2_ps[:, OUTF:OUTF + 2])

                # ---- layer-2 attention, transposed: e2T[j, i] ----
                e2sT_ps = psR.tile([1, N], F32, tag="row")
                nc.tensor.matmul(
                    e2sT_ps[:], e2c_sb[:, 0:1], ident_sb[:N, :N],
                    start=True, stop=True,
                )
                e2sT_sb = sbt.tile([1, N], F32, tag="e2sT")
                nc.vector.tensor_copy(e2sT_sb[:], e2sT_ps[:])
                e2b_sb = sbt.tile([N, N], F32, tag="e2b")
                nc.gpsimd.partition_broadcast(e2b_sb[:], e2sT_sb[:])
                # mask + e2d folded off the critical chain (same leaky
                # commute as layer 1)
                med2_sb = sbt.tile([N, N], F32, tag="med2")
                nc.vector.tensor_scalar(
                    med2_sb[:], adjmT_sb[:, 0:N], e2c_sb[:, 1:2], None, OP.add
                )
                e2_sb = sbt.tile([N, N], F32, tag="e2")
                nc.vector.tensor_add(e2_sb[:], e2b_sb[:], med2_sb[:])
                nc.vector.scalar_tensor_tensor(
                    e2_sb[:], e2_sb[:], 0.2, e2_sb[:], op0=OP.mult, op1=OP.max
                )
                u2T_sb = sbt.tile([N, N], BF16, tag="u2T")
                nc.scalar.activation(u2T_sb[:], e2_sb[:], ACTF.Exp)

                s2_ps = psR.tile([N, 1], F32, tag="row")
                nc.tensor.matmul(
                    s2_ps[:], u2T_sb[:], onescb_sb[:N, 0:1], start=True, stop=True
                )
                r2_sb = sbt.tile([N, 1], F32, tag="r2")
                nc.vector.reciprocal_approx_fast(r2_sb[:], s2_ps[:])

                # o2T = g2^T @ u2T (unnormalized); r2 folded into m below
                o2T_ps = psO.tile([OUTF, N], F32, tag="o")
                nc.tensor.matmul(
                    o2T_ps[:], g2_sb[:], u2T_sb[:], start=True, stop=True
                )
                o2T_sb = sbt.tile([OUTF, N], F32, tag="o2T")
                nc.vector.tensor_copy(o2T_sb[:], o2T_ps[:])
                m_ps = psO.tile([N, 1], F32, tag="o")
                nc.tensor.matmul(
                    m_ps[:], o2T_sb[:], onesc_sb[:OUTF, 0:1], start=True, stop=True
                )
                m_sb = sbt.tile([N, 1], F32, tag="m")
                nc.vector.tensor_scalar(
                    m_sb[:], m_ps[:], r2_sb[:, 0:1], None, OP.mult
                )

                # ---- MLP head (mw1 pre-divided by 64 on host) ----
                z1_ps = psO.tile([1, 12], F32, tag="o")
                nc.tensor.matmul(z1_ps[:], m_sb[:], mw1_sb[:], start=True, stop=True)
                z1_sb = sbt.tile([1, 12], F32, tag="z1")
                nc.vector.tensor_add(z1_sb[:], z1_ps[:], mb1_sb[:])
                zt_sb = sbt.tile([1, 12], F32, tag="zt")
                nc.vector.tensor_mul(zt_sb[:], z1_sb[:], mw2t_sb[:])
                z2_sb = sbt.tile([1, 1], F32, tag="z2")
                nc.vector.tensor_reduce(z2_sb[:], zt_sb[:], axis=AX.X, op=OP.add)
                # sigmoid(z + b) = 1/(1 + exp(-z - b)): keeps the Exp act
                # table loaded (no Sigmoid table switch)
                ez_sb = sbt.tile([1, 1], F32, tag="ez")
                nc.scalar.activation(
                    ez_sb[:], z2_sb[:], ACTF.Exp, bias=mb2n_sb[:, 0:1],
                    scale=-1.0,
                )
                ez1_sb = sbt.tile([1, 1], F32, tag="ez1")
                nc.vector.tensor_scalar(ez1_sb[:], ez_sb[:], 1.0, None, OP.add)
                res_sb = sbt.tile([1, 1], F32, tag="res")
                nc.vector.reciprocal(res_sb[:], ez1_sb[:])
                nc.sync.dma_start(out.ap(), res_sb[:])

    nc.compile()
    return nc


_NC_CACHE = []


def _get_nc():
    if not _NC_CACHE:
        _NC_CACHE.append(build())
    return _NC_CACHE[0]


def _q8(a):
    import ml_dtypes

    return np.clip(a, -240.0, 240.0).astype(ml_dtypes.float8_e4m3)


def _prep_in_maps(x, adj, W1, a1, W2, a2, mw1, mb1, mw2, mb2):
    import ml_dtypes

    # host folds: attention logit weights as extra GEMM columns
    W1r = W1.reshape(KTOT, HEADS, F1)
    Wsrc = np.einsum("khf,hf->kh", W1r, a1[:, :F1]).astype(np.float32)
    Wdst = np.einsum("khf,hf->kh", W1r, a1[:, F1:]).astype(np.float32)
    sx = 2.0 ** np.floor(np.log2(224.0 / np.abs(x).max()))
    sW = 2.0 ** np.floor(np.log2(224.0 / np.abs(W1).max()))
    sA = 2.0 ** np.floor(
        np.log2(224.0 / max(np.abs(Wsrc).max(), np.abs(Wdst).max()))
    )
    scl = np.zeros((128, 2), np.float32)
    scl[:, 0] = 1.0 / (sx * sW)
    scl[:, 1] = 1.0 / (sx * sA)

    wq = _q8(
        np.concatenate([W1 * sW, Wsrc * sA, Wdst * sA], axis=1)
    )  # [131072, 2064] fp8
    xq = _q8(x * sx)  # [46, 131072] fp8

    mask = np.where(adj[:, :, 0], np.float32(0.0), np.float32(MASK_NEG))
    adjmT = np.ascontiguousarray(np.tile(mask.T, (1, HEADS)))  # [46, 368]

    a2s, a2d = a2[0, :OUTF], a2[0, OUTF:]
    w2full = np.concatenate(
        [W2, (W2 @ a2s)[:, None], (W2 @ a2d)[:, None]], axis=1
    ).astype(ml_dtypes.bfloat16)  # [2048, 66]
    w2r = np.ascontiguousarray(
        w2full.reshape(KT2, 128, OUTF + 2).transpose(1, 0, 2).reshape(128, -1)
    )

    shared = {
        "w2b": w2r,
        "adjmT": adjmT,
        "scl": scl,
        "ident": np.eye(128, dtype=np.float32),
        "identb": np.eye(128, dtype=ml_dtypes.bfloat16),
        "onesc": np.ones((128, 1), np.float32),
        "onescb": np.ones((128, 1), ml_dtypes.bfloat16),
        "mw1": np.ascontiguousarray(mw1 / np.float32(OUTF)),
        "mb1": mb1.reshape(1, 12).astype(np.float32),
        "mw2t": np.ascontiguousarray(mw2.reshape(1, 12)),
        "mb2n": (-mb2).reshape(1, 1).astype(np.float32),
    }
    in_maps = []
    for c in range(NCORES):
        m = dict(shared)
        xc = xq[:, KC * c:KC * (c + 1)]  # [46, 16384]
        xcT = np.zeros((KT, 128, XP), xq.dtype)
        xcT[:, :, :N] = xc.T.reshape(KT, 128, N)
        m["xs"] = np.ascontiguousarray(
            xcT.transpose(1, 0, 2).reshape(128, KT * XP)
        )
        wc = wq[KC * c:KC * (c + 1), :]  # [16384, 2064]
        m["wf"] = np.ascontiguousarray(
            wc.reshape(KT, 128, WCOL).transpose(1, 0, 2).reshape(128, KT * WCOL)
        )
        in_maps.append(m)
    return in_maps


def kernel(**inputs):
    x = np.asarray(inputs["x"], dtype=np.float32)
    adj = np.asarray(inputs["adj_mat"]).astype(bool).reshape(N, N, 1)
    W1 = np.asarray(inputs["W1"], dtype=np.float32)
    a1 = np.asarray(inputs["a1"], dtype=np.float32)
    W2 = np.asarray(inputs["W2"], dtype=np.float32)
    a2 = np.asarray(inputs["a2"], dtype=np.float32)
    mw1 = np.asarray(inputs["mlp_w1"], dtype=np.float32)
    mb1 = np.asarray(inputs["mlp_b1"], dtype=np.float32)
    mw2 = np.asarray(inputs["mlp_w2"], dtype=np.float32)
    mb2 = np.asarray(inputs["mlp_b2"], dtype=np.float32)

    nc = _get_nc()
    in_maps = _prep_in_maps(x, adj, W1, a1, W2, a2, mw1, mb1, mw2, mb2)
    res = run_bass_kernel_spmd(nc, in_maps, core_ids=list(range(NCORES)))
    return res.results[0]["out"].reshape(1).astype(np.float32)
